# revision 14
# baseline (speedup 1.0000x reference)
"""GCN layer (PyG GCNConv) on 8 Trainium2 NeuronCores via Bass/Tile.

Reference computation:
    xw = x @ W; add self-loops; norm = dinv[src]*dinv[dst] with
    dinv = 1/sqrt(deg incl. self-loops); out = segment_sum(xw[src]*norm
    over dst) + b.

Device algorithm (uses linearity: W and dinv_dst commute with the sum):
    out[d] = dinv[d] * ( sum_{e: dst(e)=d} dinv[src_e] * x[src_e] ) @ W + b

Sharding: dst nodes in 128-row blocks, blocks split contiguously across 8
cores (the sharding_hint's node partition).  Each core:
  - dma_gather's the bf16 x rows of its incident edges (int16 indices into
    4 x 32768-row windows of a replicated [N,64] bf16 x table),
  - builds a selection matrix PT[e,d] = (dstloc[e]==d)*dinv_src[e] with one
    fused VectorE tensor_scalar (is_equal then mult) against an iota const,
  - accumulates aggT[64 feat, 128 dst] += M[e,f].T @ PT per dst block on
    the TensorE in PSUM across the block's edge tiles,
  - applies W with a second matmul, then dinv_dst (per-partition scalar)
    and bias, and writes its [12544, 64] f32 output shard.

Host does only integer/index prep (sort by (dst block, src window), int16
index tables, degree counts); all float tensor compute runs on device.
"""

import os
import numpy as np

try:
    import ml_dtypes

    BF16 = ml_dtypes.bfloat16
except Exception:  # pragma: no cover
    BF16 = np.float32

N = 100000
E = 1600000
ELEM128 = False
D = 64
CORES = 8
BLK = 128


def _cfg(n_nodes, n_cores, blocks_per_chunk):
    """Compile-time geometry derived from node count."""
    nblk_raw = -(-n_nodes // BLK)
    # pad so blocks divide evenly among cores
    nblk = -(-nblk_raw // n_cores) * n_cores
    npad = nblk * BLK
    bpc = nblk // n_cores
    win = 32768
    nwin = -(-npad // win)
    cb = blocks_per_chunk
    nchunk = -(-bpc // cb)
    return dict(nblk=nblk, npad=npad, bpc=bpc, win=win, nwin=nwin, cb=cb,
                nchunk=nchunk)


# ---------------------------------------------------------------------------
# Host-side preprocessing: edge grouping + device input tables
# ---------------------------------------------------------------------------

def _prep(x, edge_index, W, b, n_cores=CORES, blocks_per_chunk=14):
    n = x.shape[0]
    g = _cfg(n, n_cores, blocks_per_chunk)
    nblk, npad, bpc = g["nblk"], g["npad"], g["bpc"]
    win, nwin = g["win"], g["nwin"]

    src = np.asarray(edge_index[0], dtype=np.int64)
    dst = np.asarray(edge_index[1], dtype=np.int64)
    deg = (np.bincount(dst, minlength=n) + 1).astype(np.float32)
    dinv = 1.0 / np.sqrt(deg)  # [n] f32

    loop = np.arange(n, dtype=np.int64)
    src_all = np.concatenate([src, loop])
    dst_all = np.concatenate([dst, loop])

    blk = dst_all >> 7
    wid = src_all >> 15
    key = blk * nwin + wid
    order = np.argsort(key, kind="stable")
    src_s = src_all[order]
    dst_s = dst_all[order]

    cnt = np.bincount(key, minlength=nblk * nwin).reshape(nblk, nwin)
    ends = np.cumsum(cnt.reshape(-1)).reshape(nblk, nwin)
    starts = ends - cnt

    # tiles per (local block, window): shared across cores (max over cores)
    cnt_c = cnt.reshape(n_cores, bpc, nwin)
    tbw = -(-np.max(cnt_c, axis=0) // BLK)  # [bpc, nwin]
    tbw[:, 0] = np.maximum(tbw[:, 0], 1)   # every block has >=1 tile
    ktot = int(tbw.sum()) * BLK            # idx slots per core

    # slot offset of each (local block, window) group in the stream.
    # stream order: chunk-major -> window -> block-within-chunk -> tiles
    cb, nchunk = g["cb"], g["nchunk"]
    grp_off = np.zeros((bpc, nwin), dtype=np.int64)
    ninst = []  # (w, num_idxs, slot_offset) per chunk in order
    pos = 0
    for c in range(nchunk):
        b_lo, b_hi = c * cb, min((c + 1) * cb, bpc)
        for w in range(nwin):
            inst_off = pos
            for lb in range(b_lo, b_hi):
                grp_off[lb, w] = pos
                pos += int(tbw[lb, w]) * BLK
            ninst.append((c, w, pos - inst_off, inst_off))
    assert pos == ktot

    # per-core tables
    idx16 = np.zeros((n_cores, ktot), dtype=np.int16)
    dloc = np.zeros((n_cores, ktot), dtype=np.float32)
    dsrc = np.zeros((n_cores, ktot), dtype=np.float32)
    for m in range(n_cores):
        for lb in range(bpc):
            gb = m * bpc + lb
            for w in range(nwin):
                s, e = int(starts[gb, w]), int(ends[gb, w])
                if e == s:
                    continue
                o = int(grp_off[lb, w])
                idx16[m, o:o + e - s] = (src_s[s:e] - w * win).astype(np.int16)
                dloc[m, o:o + e - s] = (dst_s[s:e] - gb * BLK).astype(np.float32)
                dsrc[m, o:o + e - s] = dinv[src_s[s:e]]

    # device layouts.  xb has 256-byte row pitch (dma_gather stride must be
    # a multiple of 256B); only the first 64 columns hold data.
    ttot = ktot // BLK
    xb = np.zeros((npad, 2 * D), dtype=BF16)
    xb[:n, :D] = np.asarray(x, dtype=np.float32).astype(BF16)
    iota = np.broadcast_to(np.arange(BLK, dtype=np.float32), (BLK, BLK)).astype(BF16)
    bias_t = np.broadcast_to(np.asarray(b, dtype=np.float32), (BLK, D)).copy()
    dinv_pad = np.zeros(npad, dtype=np.float32)
    dinv_pad[:n] = dinv

    in_maps = []
    for m in range(n_cores):
        wrap = idx16[m].reshape(ktot // 16, 16).T  # [16, ktot/16]
        in_maps.append({
            "xb": xb,
            "w_mat": np.asarray(W, dtype=np.float32),
            "bias_t": bias_t,
            "iota": iota,
            "idxs": np.tile(wrap, (BLK // 16, 1)).copy(),
            "dstloc": dloc[m].reshape(ttot, BLK).T.copy(),
            "dinvsrc": dsrc[m].reshape(ttot, BLK).T.copy(),
            "dinv_own": dinv_pad[m * bpc * BLK:(m + 1) * bpc * BLK]
                        .reshape(bpc, BLK).T.copy(),
        })

    meta = dict(g=g, tbw=tbw, grp_off=grp_off, ninst=ninst, ktot=ktot,
                ttot=ttot, n=n, n_cores=n_cores)
    return in_maps, meta


# ---------------------------------------------------------------------------
# Bass program
# ---------------------------------------------------------------------------

def _dma_gather_small(gp, out_ap, in_ap, idxs_ap, num_idxs, elem_size, elem_step,
                      queue_num=0, single_packet=False):
    """bass.BassGpSimd.dma_gather (non-transpose, DRAM source) minus the
    `elem_size_bytes % 256 == 0` assert.  The Q7 kernel only requires the row
    *stride* to be a multiple of 256B (stride_bytes_256 descriptor field);
    the moved payload per index may be smaller.  Mirrors bass.py's
    construction of InstDMAGatherAnt."""
    import concourse.mybir as mybir
    from concourse import ap_utils
    from concourse._compat import exact_div

    assert idxs_ap.dtype == mybir.dt.int16
    assert in_ap.dtype == out_ap.dtype
    assert ap_utils.ap_is_contiguous(in_ap.ap[1:])
    assert ap_utils.ap_is_contiguous(out_ap.ap[1:])
    assert ap_utils.ap_is_contiguous(idxs_ap.ap[1:])
    assert in_ap.ap[0][0] == elem_step
    assert in_ap.ap[-1][1] == out_ap.ap[-1][1] == elem_size
    assert out_ap.ap[0][1] * out_ap.ap[1][1] == num_idxs
    stride_bytes = elem_step * mybir.dt.size(in_ap.dtype)
    stride_bytes_256 = exact_div(stride_bytes, 256)
    assert 0 < stride_bytes_256 < 256

    _in_ap = gp.lower_ap_dma(in_ap, for_custom_bir_dma=True)
    _idxs_ap = gp.lower_ap(idxs_ap)
    _out_ap = gp.lower_ap(out_ap)
    return gp.add_instruction(
        mybir.InstDMAGatherAnt(
            name=gp.bass.get_next_instruction_name(),
            ins=[*_in_ap, _idxs_ap, gp.lower_val_access(gp.to_reg(num_idxs))],
            outs=[_out_ap],
            transpose=False,
            num_idxs=num_idxs,
            elem_size=elem_size,
            stride_bytes_256=stride_bytes_256,
            gen_mode=0,
            single_packet=single_packet,
            queue_num=queue_num,
            sbuf_tokens_per_rank=0,
            sbuf_free_dim_per_rank=0,
            sbuf_free_dim_pad_per_rank=0,
            sbuf_byte_offset=0,
        )
    )


def _build(meta):
    import concourse.bacc as bacc
    import concourse.mybir as mybir
    import concourse.tile as tile

    g = meta["g"]
    nblk, npad, bpc = g["nblk"], g["npad"], g["bpc"]
    win, nwin, cb, nchunk = g["win"], g["nwin"], g["cb"], g["nchunk"]
    tbw, grp_off, ninst = meta["tbw"], meta["grp_off"], meta["ninst"]
    ktot, ttot = meta["ktot"], meta["ttot"]

    f32 = mybir.dt.float32
    bf16 = mybir.dt.bfloat16
    i16 = mybir.dt.int16

    nc = bacc.Bacc("TRN2", target_bir_lowering=False, debug=False,
                   num_swdge_queues=4)

    xb = nc.dram_tensor("xb", [npad, 2 * D], bf16, kind="ExternalInput")
    w_mat = nc.dram_tensor("w_mat", [D, D], f32, kind="ExternalInput")
    bias_t = nc.dram_tensor("bias_t", [BLK, D], f32, kind="ExternalInput")
    iota_d = nc.dram_tensor("iota", [BLK, BLK], bf16, kind="ExternalInput")
    idxs_d = nc.dram_tensor("idxs", [BLK, ktot // 16], i16, kind="ExternalInput")
    dstloc_d = nc.dram_tensor("dstloc", [BLK, ttot], f32, kind="ExternalInput")
    dinvsrc_d = nc.dram_tensor("dinvsrc", [BLK, ttot], f32, kind="ExternalInput")
    dinv_own_d = nc.dram_tensor("dinv_own", [BLK, bpc], f32, kind="ExternalInput")
    out_d = nc.dram_tensor("out", [bpc * BLK, D], f32, kind="ExternalOutput")

    with tile.TileContext(nc) as tc:
        with (
            tc.tile_pool(name="const", bufs=1) as cpool,
            tc.tile_pool(name="mbuf", bufs=2) as mpool,
            tc.tile_pool(name="pt", bufs=8) as ptpool,
            tc.tile_pool(name="agg", bufs=4) as aggpool,
            tc.tile_pool(name="ob", bufs=4) as obpool,
            tc.tile_pool(name="ps1", bufs=2, space="PSUM") as ps1pool,
            tc.tile_pool(name="ps2", bufs=2, space="PSUM") as ps2pool,
        ):
            w_sb = cpool.tile([D, D], f32, tag="w")
            nc.sync.dma_start(out=w_sb[:], in_=w_mat[:])
            bias_sb = cpool.tile([BLK, D], f32, tag="bias")
            nc.sync.dma_start(out=bias_sb[:], in_=bias_t[:])
            iota_sb = cpool.tile([BLK, BLK], bf16, tag="iota")
            nc.sync.dma_start(out=iota_sb[:], in_=iota_d[:])
            idxs_sb = cpool.tile([BLK, ktot // 16], i16, tag="idxs")
            nc.sync.dma_start(out=idxs_sb[:], in_=idxs_d[:])
            dloc_sb = cpool.tile([BLK, ttot], f32, tag="dloc")
            nc.sync.dma_start(out=dloc_sb[:], in_=dstloc_d[:])
            dsrc_sb = cpool.tile([BLK, ttot], f32, tag="dsrc")
            nc.sync.dma_start(out=dsrc_sb[:], in_=dinvsrc_d[:])
            dinv_sb = cpool.tile([BLK, bpc], f32, tag="dinv")
            nc.sync.dma_start(out=dinv_sb[:], in_=dinv_own_d[:])

            # window row counts in the xb table
            wrows = [min(win, npad - w * win) for w in range(nwin)]

            inst_by_chunk = {}
            for (c, w, num_idxs, off) in ninst:
                inst_by_chunk.setdefault(c, []).append((w, num_idxs, off))

            # dma_gather descriptor budget: one desc per 16 idxs per engine;
            # cap each instruction well under the SWDGE ring capacity.
            GCAP = 3456
            qn = [0]

            def emit_gather(mt, w, off, num_idxs, es, row_lo):
                pos = 0
                while pos < num_idxs:
                    ni = min(GCAP, num_idxs - pos)
                    o = off + pos
                    _dma_gather_small(
                        nc.gpsimd,
                        mt[:, pos // BLK:(pos + ni) // BLK, :],
                        xb[w * win:w * win + wrows[w], row_lo:row_lo + es],
                        idxs_sb[:, o // 16:(o + ni) // 16],
                        ni,
                        es,
                        2 * D,
                        queue_num=qn[0] % 4,
                        single_packet=False,
                    )
                    qn[0] += 1
                    pos += ni

            for c in range(nchunk):
                mtiles = {}
                for (w, num_idxs, off) in inst_by_chunk[c]:
                    if num_idxs == 0:
                        continue
                    t_cw = num_idxs // BLK
                    if ELEM128:
                        mt = mpool.tile([BLK, t_cw, 2 * D], bf16, tag=f"m{w}")
                        emit_gather(mt, w, off, num_idxs, 2 * D, 0)
                    else:
                        mt = mpool.tile([BLK, t_cw, D], bf16, tag=f"m{w}")
                        emit_gather(mt, w, off, num_idxs, D, 0)
                    mtiles[w] = (mt, off)

                b_lo, b_hi = c * cb, min((c + 1) * cb, bpc)
                for lb in range(b_lo, b_hi):
                    tb = int(tbw[lb].sum())
                    ps = ps1pool.tile([D, BLK], f32, tag="agg")
                    k = 0
                    for w in range(nwin):
                        for t in range(int(tbw[lb, w])):
                            mt, moff = mtiles[w]
                            gt = (grp_off[lb, w] - moff) // BLK + t
                            col = grp_off[lb, w] // BLK + t
                            pt = ptpool.tile([BLK, BLK], bf16, tag="pt")
                            nc.vector.tensor_scalar(
                                pt[:],
                                iota_sb[:],
                                dloc_sb[:, col:col + 1],
                                dsrc_sb[:, col:col + 1],
                                mybir.AluOpType.is_equal,
                                mybir.AluOpType.mult,
                            )
                            nc.tensor.matmul(
                                ps[:],
                                mt[:, gt, 0:D],
                                pt[:],
                                start=(k == 0),
                                stop=(k == tb - 1),
                            )
                            k += 1
                    aggt = aggpool.tile([D, BLK], f32, tag="aggt")
                    nc.vector.tensor_copy(out=aggt[:], in_=ps[:])
                    ps2 = ps2pool.tile([BLK, D], f32, tag="o2")
                    nc.tensor.matmul(ps2[:], aggt[:], w_sb[:], start=True, stop=True)
                    ob = obpool.tile([BLK, D], f32, tag="ob")
                    nc.vector.tensor_scalar_mul(ob[:], ps2[:], dinv_sb[:, lb:lb + 1])
                    nc.vector.tensor_add(out=ob[:], in0=ob[:], in1=bias_sb[:])
                    nc.sync.dma_start(out=out_d[lb * BLK:(lb + 1) * BLK, :], in_=ob[:])
    return nc


# ---------------------------------------------------------------------------
# Entry points
# ---------------------------------------------------------------------------

def _install_ntff_hook_shim():
    """The agent image's antenv package lacks axon_hooks; provide it so
    run_bass_kernel_spmd(trace=True) can reach the NTFF profiler via the
    ctypes hook that trn_agent_boot carries."""
    import sys
    import types

    try:
        import antenv.axon_hooks  # noqa: F401
        return
    except ImportError:
        pass
    try:
        from trn_agent_boot.trn_boot import _ntff_profile_via_ctypes

        hook = _ntff_profile_via_ctypes("/opt/axon/libaxon_pjrt.so")
    except Exception:
        hook = None
    mod = types.ModuleType("antenv.axon_hooks")
    mod.get_axon_ntff_profile_hook = lambda: hook
    mod.set_axon_ntff_profile_hook = lambda h: None
    sys.modules["antenv.axon_hooks"] = mod


def _run(x, edge_index, W, b, trace=False):
    from concourse.bass_utils import run_bass_kernel_spmd

    if trace:
        _install_ntff_hook_shim()

    in_maps, meta = _prep(x, edge_index, W, b)
    nc = _build(meta)
    nc.compile()
    res = run_bass_kernel_spmd(
        nc, in_maps, list(range(meta["n_cores"])), trace=trace
    )
    n, n_cores = meta["n"], meta["n_cores"]
    shards = [res.results[m]["out"] for m in range(n_cores)]
    out = np.concatenate(shards, axis=0)[:n].astype(np.float32)
    return out, res


def kernel(x, edge_index, W, b):
    out, _ = _run(x, edge_index, W, b, trace=False)
    return out


def kernel_profiled(x, edge_index, W, b):
    out, res = _run(x, edge_index, W, b, trace=True)
    return out, res


# revision 31
# speedup vs baseline: 1.5998x; 1.5998x over previous
"""GCN layer (PyG GCNConv) on 8 Trainium2 NeuronCores via Bass/Tile.

Reference computation:
    xw = x @ W; add self-loops; norm = dinv[src]*dinv[dst] with
    dinv = 1/sqrt(deg incl. self-loops); out = segment_sum(xw[src]*norm
    over dst) + b.

Device algorithm (uses linearity: W and dinv_dst commute with the sum):
    out[d] = dinv[d] * ( sum_{e: dst(e)=d} dinv[src_e] * x[src_e] ) @ W + b

Sharding: dst nodes in 128-row blocks; blocks are dealt to (core, slot)
by similar per-window size profiles so the SPMD per-slot tile caps (max
over cores) waste little padding.  Each core:
  - dma_gather's rows of a dinv-prescaled bf16 x table for its incident
    edges (int16 indices into 4 x 32768-row windows; 256B row pitch with
    128B payload; multi-packet gathers capped under the SWDGE ring size,
    round-robined over 4 queues),
  - builds one-hot selection tiles PT[e,d] = (dstloc[e]==d) on the
    VectorE, batched 4 tiles per tensor_tensor against a repeated iota
    const with a stride-0 broadcast of the dstloc columns,
  - accumulates aggT[64 feat, 128 dst] += M.T @ PT per dst block on the
    TensorE in PSUM across the block's edge tiles,
  - applies W with a second matmul, then dinv_dst and bias on the
    Vector/Scalar engines, and writes its [12544, 64] f32 output shard.

Host does only integer/index prep (degree counts, sort by (dst block,
src window), int16 index tables, dinv row prescale); the O(E) message
gather/aggregation and all matmuls run on device.
"""

import os
import numpy as np

try:
    import ml_dtypes

    BF16 = ml_dtypes.bfloat16
except Exception:  # pragma: no cover
    BF16 = np.float32

N = 100000
E = 1600000
ELEM128 = False
PTB = 4  # PT tiles built per DVE op
D = 64
CORES = 8
BLK = 128


def _cfg(n_nodes, n_cores, blocks_per_chunk):
    """Compile-time geometry derived from node count."""
    nblk_raw = -(-n_nodes // BLK)
    # pad so blocks divide evenly among cores
    nblk = -(-nblk_raw // n_cores) * n_cores
    npad = nblk * BLK
    bpc = nblk // n_cores
    win = 32768
    nwin = -(-npad // win)
    cb = blocks_per_chunk
    nchunk = -(-bpc // cb)
    return dict(nblk=nblk, npad=npad, bpc=bpc, win=win, nwin=nwin, cb=cb,
                nchunk=nchunk)


# ---------------------------------------------------------------------------
# Host-side preprocessing: edge grouping + device input tables
# ---------------------------------------------------------------------------

def _prep(x, edge_index, W, b, n_cores=CORES, blocks_per_chunk=14):
    n = x.shape[0]
    g = _cfg(n, n_cores, blocks_per_chunk)
    nblk, npad, bpc = g["nblk"], g["npad"], g["bpc"]
    win, nwin = g["win"], g["nwin"]

    src = np.asarray(edge_index[0], dtype=np.int64)
    dst = np.asarray(edge_index[1], dtype=np.int64)
    deg = (np.bincount(dst, minlength=n) + 1).astype(np.float32)
    dinv = 1.0 / np.sqrt(deg)  # [n] f32

    loop = np.arange(n, dtype=np.int64)
    src_all = np.concatenate([src, loop])
    dst_all = np.concatenate([dst, loop])

    blk = dst_all >> 7
    wid = src_all >> 15
    key = blk * nwin + wid
    order = np.argsort(key, kind="stable")
    src_s = src_all[order]
    dst_s = dst_all[order]

    cnt = np.bincount(key, minlength=nblk * nwin).reshape(nblk, nwin)
    ends = np.cumsum(cnt.reshape(-1)).reshape(nblk, nwin)
    starts = ends - cnt

    # Balance: assign blocks to (core, slot) so the 8 blocks sharing a slot
    # have matching per-window tile counts (the per-slot cap is the max over
    # cores, so grouping blocks with equal ceil-profiles minimizes padding).
    prof = -(-cnt // BLK)
    keys = [cnt.sum(axis=1)] + [prof[:, w] for w in range(nwin - 1, -1, -1)]
    order_b = np.lexsort(tuple(keys))
    perm = np.empty((n_cores, bpc), dtype=np.int64)
    for s in range(bpc):
        perm[:, s] = order_b[s * n_cores:(s + 1) * n_cores]

    # tiles per (slot, window): shared across cores (max over cores)
    cnt_s = cnt[perm]                       # [n_cores, bpc, nwin]
    tbw = -(-np.max(cnt_s, axis=0) // BLK)  # [bpc, nwin]
    tbw[:, 0] = np.maximum(tbw[:, 0], 1)   # every slot has >=1 tile
    ktot = int(tbw.sum()) * BLK            # idx slots per core

    # slot offset of each (local block, window) group in the stream.
    # stream order: chunk-major -> window -> block-within-chunk -> tiles
    cb, nchunk = g["cb"], g["nchunk"]
    grp_off = np.zeros((bpc, nwin), dtype=np.int64)
    ninst = []  # (w, num_idxs, slot_offset) per chunk in order
    pos = 0
    for c in range(nchunk):
        b_lo, b_hi = c * cb, min((c + 1) * cb, bpc)
        for w in range(nwin):
            inst_off = pos
            for lb in range(b_lo, b_hi):
                grp_off[lb, w] = pos
                pos += int(tbw[lb, w]) * BLK
            ninst.append((c, w, pos - inst_off, inst_off))
    assert pos == ktot

    # per-core tables.  dloc pads are 200 -> one-hot column is all-zero,
    # so pad slots contribute nothing regardless of gathered data.
    idx16 = np.zeros((n_cores, ktot), dtype=np.int16)
    dloc = np.full((n_cores, ktot), 200.0, dtype=np.float32)
    for m in range(n_cores):
        for lb in range(bpc):
            gb = int(perm[m, lb])
            for w in range(nwin):
                s, e = int(starts[gb, w]), int(ends[gb, w])
                if e == s:
                    continue
                o = int(grp_off[lb, w])
                idx16[m, o:o + e - s] = (src_s[s:e] - w * win).astype(np.int16)
                dloc[m, o:o + e - s] = (dst_s[s:e] - gb * BLK).astype(np.float32)

    # device layouts.  xb has 256-byte row pitch (dma_gather stride must be
    # a multiple of 256B); only the first 64 columns hold data.  Rows are
    # pre-scaled by dinv[src] so the gathered messages carry the source-side
    # normalization (W and dinv[dst] are applied after aggregation).
    ttot = ktot // BLK
    xb = np.zeros((npad, 2 * D), dtype=BF16)
    xb[:n, :D] = (np.asarray(x, dtype=np.float32)
                  * dinv[:, None]).astype(BF16)
    iota = np.tile(np.arange(BLK, dtype=np.float32),
                   (BLK, PTB)).astype(BF16)
    bias_t = np.broadcast_to(np.asarray(b, dtype=np.float32), (BLK, D)).copy()
    dinv_pad = np.zeros(npad, dtype=np.float32)
    dinv_pad[:n] = dinv
    dinv_blk = dinv_pad.reshape(nblk, BLK)

    in_maps = []
    for m in range(n_cores):
        wrap = idx16[m].reshape(ktot // 16, 16).T  # [16, ktot/16]
        in_maps.append({
            "xb": xb,
            "w_mat": np.asarray(W, dtype=np.float32),
            "bias_t": bias_t,
            "iota": iota,
            "idxs": np.tile(wrap, (BLK // 16, 1)).copy(),
            "dstloc": dloc[m].reshape(ttot, BLK).T.astype(BF16).copy(),
            "dinv_own": dinv_blk[perm[m]].T.copy(),
        })

    meta = dict(g=g, tbw=tbw, grp_off=grp_off, ninst=ninst, ktot=ktot,
                ttot=ttot, n=n, n_cores=n_cores, perm=perm)
    return in_maps, meta


# ---------------------------------------------------------------------------
# Bass program
# ---------------------------------------------------------------------------

def _dma_gather_small(gp, out_ap, in_ap, idxs_ap, num_idxs, elem_size, elem_step,
                      queue_num=0, single_packet=False):
    """bass.BassGpSimd.dma_gather (non-transpose, DRAM source) minus the
    `elem_size_bytes % 256 == 0` assert.  The Q7 kernel only requires the row
    *stride* to be a multiple of 256B (stride_bytes_256 descriptor field);
    the moved payload per index may be smaller.  Mirrors bass.py's
    construction of InstDMAGatherAnt."""
    import concourse.mybir as mybir
    from concourse import ap_utils
    from concourse._compat import exact_div

    assert idxs_ap.dtype == mybir.dt.int16
    assert in_ap.dtype == out_ap.dtype
    assert ap_utils.ap_is_contiguous(in_ap.ap[1:])
    assert ap_utils.ap_is_contiguous(out_ap.ap[1:])
    assert ap_utils.ap_is_contiguous(idxs_ap.ap[1:])
    assert in_ap.ap[0][0] == elem_step
    assert in_ap.ap[-1][1] == out_ap.ap[-1][1] == elem_size
    assert out_ap.ap[0][1] * out_ap.ap[1][1] == num_idxs
    stride_bytes = elem_step * mybir.dt.size(in_ap.dtype)
    stride_bytes_256 = exact_div(stride_bytes, 256)
    assert 0 < stride_bytes_256 < 256

    _in_ap = gp.lower_ap_dma(in_ap, for_custom_bir_dma=True)
    _idxs_ap = gp.lower_ap(idxs_ap)
    _out_ap = gp.lower_ap(out_ap)
    return gp.add_instruction(
        mybir.InstDMAGatherAnt(
            name=gp.bass.get_next_instruction_name(),
            ins=[*_in_ap, _idxs_ap, gp.lower_val_access(gp.to_reg(num_idxs))],
            outs=[_out_ap],
            transpose=False,
            num_idxs=num_idxs,
            elem_size=elem_size,
            stride_bytes_256=stride_bytes_256,
            gen_mode=0,
            single_packet=single_packet,
            queue_num=queue_num,
            sbuf_tokens_per_rank=0,
            sbuf_free_dim_per_rank=0,
            sbuf_free_dim_pad_per_rank=0,
            sbuf_byte_offset=0,
        )
    )


def _build(meta):
    import concourse.bacc as bacc
    import concourse.mybir as mybir
    import concourse.tile as tile

    g = meta["g"]
    nblk, npad, bpc = g["nblk"], g["npad"], g["bpc"]
    win, nwin, cb, nchunk = g["win"], g["nwin"], g["cb"], g["nchunk"]
    tbw, grp_off, ninst = meta["tbw"], meta["grp_off"], meta["ninst"]
    ktot, ttot = meta["ktot"], meta["ttot"]

    f32 = mybir.dt.float32
    bf16 = mybir.dt.bfloat16
    i16 = mybir.dt.int16

    nc = bacc.Bacc("TRN2", target_bir_lowering=False, debug=False,
                   num_swdge_queues=4)

    xb = nc.dram_tensor("xb", [npad, 2 * D], bf16, kind="ExternalInput")
    w_mat = nc.dram_tensor("w_mat", [D, D], f32, kind="ExternalInput")
    bias_t = nc.dram_tensor("bias_t", [BLK, D], f32, kind="ExternalInput")
    iota_d = nc.dram_tensor("iota", [BLK, PTB * BLK], bf16, kind="ExternalInput")
    idxs_d = nc.dram_tensor("idxs", [BLK, ktot // 16], i16, kind="ExternalInput")
    dstloc_d = nc.dram_tensor("dstloc", [BLK, ttot], bf16, kind="ExternalInput")
    dinv_own_d = nc.dram_tensor("dinv_own", [BLK, bpc], f32, kind="ExternalInput")
    out_d = nc.dram_tensor("out", [bpc * BLK, D], f32, kind="ExternalOutput")

    with tile.TileContext(nc) as tc:
        with (
            tc.tile_pool(name="const", bufs=1) as cpool,
            tc.tile_pool(name="mbuf", bufs=2) as mpool,
            tc.tile_pool(name="pt", bufs=8) as ptpool,
            tc.tile_pool(name="agg", bufs=4) as aggpool,
            tc.tile_pool(name="ob", bufs=4) as obpool,
            tc.tile_pool(name="ps1", bufs=2, space="PSUM") as ps1pool,
            tc.tile_pool(name="ps2", bufs=2, space="PSUM") as ps2pool,
        ):
            w_sb = cpool.tile([D, D], f32, tag="w")
            nc.sync.dma_start(out=w_sb[:], in_=w_mat[:])
            bias_sb = cpool.tile([BLK, D], f32, tag="bias")
            nc.sync.dma_start(out=bias_sb[:], in_=bias_t[:])
            iota_sb = cpool.tile([BLK, PTB * BLK], bf16, tag="iota")
            nc.sync.dma_start(out=iota_sb[:], in_=iota_d[:])
            idxs_sb = cpool.tile([BLK, ktot // 16], i16, tag="idxs")
            nc.sync.dma_start(out=idxs_sb[:], in_=idxs_d[:])
            dloc_sb = cpool.tile([BLK, ttot], bf16, tag="dloc")
            nc.sync.dma_start(out=dloc_sb[:], in_=dstloc_d[:])
            dinv_sb = cpool.tile([BLK, bpc], f32, tag="dinv")
            nc.sync.dma_start(out=dinv_sb[:], in_=dinv_own_d[:])

            # window row counts in the xb table
            wrows = [min(win, npad - w * win) for w in range(nwin)]

            inst_by_chunk = {}
            for (c, w, num_idxs, off) in ninst:
                inst_by_chunk.setdefault(c, []).append((w, num_idxs, off))

            # dma_gather descriptor budget: one desc per 16 idxs per engine;
            # cap each instruction well under the SWDGE ring capacity.
            GCAP = 3456
            qn = [0]

            def emit_gather(mt, w, off, num_idxs, es, row_lo):
                pos = 0
                while pos < num_idxs:
                    ni = min(GCAP, num_idxs - pos)
                    o = off + pos
                    _dma_gather_small(
                        nc.gpsimd,
                        mt[:, pos // BLK:(pos + ni) // BLK, :],
                        xb[w * win:w * win + wrows[w], row_lo:row_lo + es],
                        idxs_sb[:, o // 16:(o + ni) // 16],
                        ni,
                        es,
                        2 * D,
                        queue_num=qn[0] % 4,
                        single_packet=False,
                    )
                    qn[0] += 1
                    pos += ni

            for c in range(nchunk):
                mtiles = {}
                for (w, num_idxs, off) in inst_by_chunk[c]:
                    if num_idxs == 0:
                        continue
                    t_cw = num_idxs // BLK
                    if ELEM128:
                        mt = mpool.tile([BLK, t_cw, 2 * D], bf16, tag=f"m{w}")
                        emit_gather(mt, w, off, num_idxs, 2 * D, 0)
                    else:
                        mt = mpool.tile([BLK, t_cw, D], bf16, tag=f"m{w}")
                        emit_gather(mt, w, off, num_idxs, D, 0)
                    mtiles[w] = (mt, off)

                b_lo, b_hi = c * cb, min((c + 1) * cb, bpc)
                for lb in range(b_lo, b_hi):
                    tb = int(tbw[lb].sum())
                    ps = ps1pool.tile([D, BLK], f32, tag="agg")
                    k = 0
                    for w in range(nwin):
                        tB = int(tbw[lb, w])
                        col_b = grp_off[lb, w] // BLK
                        mt, moff = mtiles[w]
                        gt_b = (grp_off[lb, w] - moff) // BLK
                        for t0 in range(0, tB, PTB):
                            nb = min(PTB, tB - t0)
                            pt = ptpool.tile([BLK, PTB * BLK], bf16, tag="pt")
                            if nb == 1:
                                bc = (dloc_sb[:, col_b + t0:col_b + t0 + 1]
                                      .to_broadcast([BLK, BLK]))
                            else:
                                bc = (dloc_sb[:, col_b + t0:col_b + t0 + nb]
                                      .to_broadcast([BLK, nb, BLK]))
                            nc.vector.tensor_tensor(
                                out=pt[:, 0:nb * BLK],
                                in0=iota_sb[:, 0:nb * BLK],
                                in1=bc,
                                op=mybir.AluOpType.is_equal,
                            )
                            for j in range(nb):
                                nc.tensor.matmul(
                                    ps[:],
                                    mt[:, gt_b + t0 + j, 0:D],
                                    pt[:, j * BLK:(j + 1) * BLK],
                                    start=(k == 0),
                                    stop=(k == tb - 1),
                                )
                                k += 1
                    aggt = aggpool.tile([D, BLK], f32, tag="aggt")
                    nc.scalar.copy(out=aggt[:], in_=ps[:])
                    ps2 = ps2pool.tile([BLK, D], f32, tag="o2")
                    nc.tensor.matmul(ps2[:], aggt[:], w_sb[:], start=True, stop=True)
                    ob = obpool.tile([BLK, D], f32, tag="ob")
                    nc.vector.tensor_tensor(
                        out=ob[:], in0=ps2[:],
                        in1=dinv_sb[:, lb:lb + 1].to_broadcast([BLK, D]),
                        op=mybir.AluOpType.mult,
                    )
                    nc.vector.tensor_add(out=ob[:], in0=ob[:], in1=bias_sb[:])
                    nc.sync.dma_start(out=out_d[lb * BLK:(lb + 1) * BLK, :], in_=ob[:])
    return nc


# ---------------------------------------------------------------------------
# Entry points
# ---------------------------------------------------------------------------

def _install_ntff_hook_shim():
    """The agent image's antenv package lacks axon_hooks; provide it so
    run_bass_kernel_spmd(trace=True) can reach the NTFF profiler via the
    ctypes hook that trn_agent_boot carries."""
    import sys
    import types

    try:
        import antenv.axon_hooks  # noqa: F401
        return
    except ImportError:
        pass
    try:
        from trn_agent_boot.trn_boot import _ntff_profile_via_ctypes

        hook = _ntff_profile_via_ctypes("/opt/axon/libaxon_pjrt.so")
    except Exception:
        hook = None
    mod = types.ModuleType("antenv.axon_hooks")
    mod.get_axon_ntff_profile_hook = lambda: hook
    mod.set_axon_ntff_profile_hook = lambda h: None
    sys.modules["antenv.axon_hooks"] = mod


def _run(x, edge_index, W, b, trace=False):
    from concourse.bass_utils import run_bass_kernel_spmd

    if trace:
        _install_ntff_hook_shim()

    in_maps, meta = _prep(x, edge_index, W, b)
    nc = _build(meta)
    nc.compile()
    res = run_bass_kernel_spmd(
        nc, in_maps, list(range(meta["n_cores"])), trace=trace
    )
    n, n_cores = meta["n"], meta["n_cores"]
    g = meta["g"]
    perm = meta["perm"]
    out = np.empty((g["nblk"], BLK, D), dtype=np.float32)
    for m in range(n_cores):
        out[perm[m]] = res.results[m]["out"].reshape(g["bpc"], BLK, D)
    return out.reshape(-1, D)[:n].astype(np.float32), res


def kernel(x, edge_index, W, b):
    out, _ = _run(x, edge_index, W, b, trace=False)
    return out


def kernel_profiled(x, edge_index, W, b):
    out, res = _run(x, edge_index, W, b, trace=True)
    return out, res


# revision 32
# speedup vs baseline: 1.6366x; 1.0230x over previous
"""GCN layer (PyG GCNConv) on 8 Trainium2 NeuronCores via Bass/Tile.

Reference computation:
    xw = x @ W; add self-loops; norm = dinv[src]*dinv[dst] with
    dinv = 1/sqrt(deg incl. self-loops); out = segment_sum(xw[src]*norm
    over dst) + b.

Device algorithm (uses linearity: W and dinv_dst commute with the sum):
    out[d] = dinv[d] * ( sum_{e: dst(e)=d} dinv[src_e] * x[src_e] ) @ W + b

Sharding: dst nodes in 128-row blocks; blocks are dealt to (core, slot)
by similar per-window size profiles so the SPMD per-slot tile caps (max
over cores) waste little padding.  Each core:
  - dma_gather's rows of a dinv-prescaled bf16 x table for its incident
    edges (int16 indices into 4 x 32768-row windows; 256B row pitch with
    128B payload; multi-packet gathers capped under the SWDGE ring size,
    round-robined over 4 queues),
  - builds one-hot selection tiles PT[e,d] = (dstloc[e]==d) on the
    VectorE, batched 4 tiles per tensor_tensor against a repeated iota
    const with a stride-0 broadcast of the dstloc columns,
  - accumulates aggT[64 feat, 128 dst] += M.T @ PT per dst block on the
    TensorE in PSUM across the block's edge tiles,
  - applies W with a second matmul, then dinv_dst and bias on the
    Vector/Scalar engines, and writes its [12544, 64] f32 output shard.

Host does only integer/index prep (degree counts, sort by (dst block,
src window), int16 index tables, dinv row prescale); the O(E) message
gather/aggregation and all matmuls run on device.
"""

import os
import numpy as np

try:
    import ml_dtypes

    BF16 = ml_dtypes.bfloat16
except Exception:  # pragma: no cover
    BF16 = np.float32

N = 100000
E = 1600000
ELEM128 = False
PTB = 4  # PT tiles built per DVE op
D = 64
CORES = 8
BLK = 128


def _cfg(n_nodes, n_cores, blocks_per_chunk):
    """Compile-time geometry derived from node count."""
    nblk_raw = -(-n_nodes // BLK)
    # pad so blocks divide evenly among cores
    nblk = -(-nblk_raw // n_cores) * n_cores
    npad = nblk * BLK
    bpc = nblk // n_cores
    win = 32768
    nwin = -(-npad // win)
    cb = blocks_per_chunk
    nchunk = -(-bpc // cb)
    return dict(nblk=nblk, npad=npad, bpc=bpc, win=win, nwin=nwin, cb=cb,
                nchunk=nchunk)


# ---------------------------------------------------------------------------
# Host-side preprocessing: edge grouping + device input tables
# ---------------------------------------------------------------------------

def _prep(x, edge_index, W, b, n_cores=CORES, blocks_per_chunk=14):
    n = x.shape[0]
    g = _cfg(n, n_cores, blocks_per_chunk)
    nblk, npad, bpc = g["nblk"], g["npad"], g["bpc"]
    win, nwin = g["win"], g["nwin"]

    src = np.asarray(edge_index[0], dtype=np.int64)
    dst = np.asarray(edge_index[1], dtype=np.int64)
    deg = (np.bincount(dst, minlength=n) + 1).astype(np.float32)
    dinv = 1.0 / np.sqrt(deg)  # [n] f32

    loop = np.arange(n, dtype=np.int64)
    src_all = np.concatenate([src, loop])
    dst_all = np.concatenate([dst, loop])

    blk = dst_all >> 7
    wid = src_all >> 15
    key = blk * nwin + wid
    order = np.argsort(key, kind="stable")
    src_s = src_all[order]
    dst_s = dst_all[order]

    cnt = np.bincount(key, minlength=nblk * nwin).reshape(nblk, nwin)
    ends = np.cumsum(cnt.reshape(-1)).reshape(nblk, nwin)
    starts = ends - cnt

    # Balance: assign blocks to (core, slot) so the 8 blocks sharing a slot
    # have matching per-window tile counts (the per-slot cap is the max over
    # cores, so grouping blocks with equal ceil-profiles minimizes padding).
    prof = -(-cnt // BLK)
    keys = [cnt.sum(axis=1)] + [prof[:, w] for w in range(nwin - 1, -1, -1)]
    order_b = np.lexsort(tuple(keys))
    perm = np.empty((n_cores, bpc), dtype=np.int64)
    for s in range(bpc):
        perm[:, s] = order_b[s * n_cores:(s + 1) * n_cores]

    # tiles per (slot, window): shared across cores (max over cores)
    cnt_s = cnt[perm]                       # [n_cores, bpc, nwin]
    tbw = -(-np.max(cnt_s, axis=0) // BLK)  # [bpc, nwin]
    tbw[:, 0] = np.maximum(tbw[:, 0], 1)   # every slot has >=1 tile
    ktot = int(tbw.sum()) * BLK            # idx slots per core

    # slot offset of each (local block, window) group in the stream.
    # stream order: chunk-major -> window -> block-within-chunk -> tiles
    cb, nchunk = g["cb"], g["nchunk"]
    grp_off = np.zeros((bpc, nwin), dtype=np.int64)
    ninst = []  # (w, num_idxs, slot_offset) per chunk in order
    pos = 0
    for c in range(nchunk):
        b_lo, b_hi = c * cb, min((c + 1) * cb, bpc)
        for w in range(nwin):
            inst_off = pos
            for lb in range(b_lo, b_hi):
                grp_off[lb, w] = pos
                pos += int(tbw[lb, w]) * BLK
            ninst.append((c, w, pos - inst_off, inst_off))
    assert pos == ktot

    # per-core tables.  dloc pads are 200 -> one-hot column is all-zero,
    # so pad slots contribute nothing regardless of gathered data.
    idx16 = np.zeros((n_cores, ktot), dtype=np.int16)
    dloc = np.full((n_cores, ktot), 200.0, dtype=np.float32)
    for m in range(n_cores):
        for lb in range(bpc):
            gb = int(perm[m, lb])
            for w in range(nwin):
                s, e = int(starts[gb, w]), int(ends[gb, w])
                if e == s:
                    continue
                o = int(grp_off[lb, w])
                idx16[m, o:o + e - s] = (src_s[s:e] - w * win).astype(np.int16)
                dloc[m, o:o + e - s] = (dst_s[s:e] - gb * BLK).astype(np.float32)

    # device layouts.  xb has 256-byte row pitch (dma_gather stride must be
    # a multiple of 256B); only the first 64 columns hold data.  Rows are
    # pre-scaled by dinv[src] so the gathered messages carry the source-side
    # normalization (W and dinv[dst] are applied after aggregation).
    ttot = ktot // BLK
    xb = np.zeros((npad, 2 * D), dtype=BF16)
    xb[:n, :D] = (np.asarray(x, dtype=np.float32)
                  * dinv[:, None]).astype(BF16)
    iota = np.tile(np.arange(BLK, dtype=np.float32),
                   (BLK, PTB)).astype(BF16)
    bias_t = np.broadcast_to(np.asarray(b, dtype=np.float32), (BLK, D)).copy()
    dinv_pad = np.zeros(npad, dtype=np.float32)
    dinv_pad[:n] = dinv
    dinv_blk = dinv_pad.reshape(nblk, BLK)

    in_maps = []
    for m in range(n_cores):
        wrap = idx16[m].reshape(ktot // 16, 16).T  # [16, ktot/16]
        in_maps.append({
            "xb": xb,
            "w_mat": np.asarray(W, dtype=np.float32),
            "bias_t": bias_t,
            "iota": iota,
            "idxs": np.tile(wrap, (BLK // 16, 1)).copy(),
            "dstloc": dloc[m].reshape(ttot, BLK).T.astype(BF16).copy(),
            "dinv_own": dinv_blk[perm[m]].T.copy(),
        })

    meta = dict(g=g, tbw=tbw, grp_off=grp_off, ninst=ninst, ktot=ktot,
                ttot=ttot, n=n, n_cores=n_cores, perm=perm)
    return in_maps, meta


# ---------------------------------------------------------------------------
# Bass program
# ---------------------------------------------------------------------------

def _dma_gather_small(gp, out_ap, in_ap, idxs_ap, num_idxs, elem_size, elem_step,
                      queue_num=0, single_packet=False):
    """bass.BassGpSimd.dma_gather (non-transpose, DRAM source) minus the
    `elem_size_bytes % 256 == 0` assert.  The Q7 kernel only requires the row
    *stride* to be a multiple of 256B (stride_bytes_256 descriptor field);
    the moved payload per index may be smaller.  Mirrors bass.py's
    construction of InstDMAGatherAnt."""
    import concourse.mybir as mybir
    from concourse import ap_utils
    from concourse._compat import exact_div

    assert idxs_ap.dtype == mybir.dt.int16
    assert in_ap.dtype == out_ap.dtype
    assert ap_utils.ap_is_contiguous(in_ap.ap[1:])
    assert ap_utils.ap_is_contiguous(out_ap.ap[1:])
    assert ap_utils.ap_is_contiguous(idxs_ap.ap[1:])
    assert in_ap.ap[0][0] == elem_step
    assert in_ap.ap[-1][1] == out_ap.ap[-1][1] == elem_size
    assert out_ap.ap[0][1] * out_ap.ap[1][1] == num_idxs
    stride_bytes = elem_step * mybir.dt.size(in_ap.dtype)
    stride_bytes_256 = exact_div(stride_bytes, 256)
    assert 0 < stride_bytes_256 < 256

    _in_ap = gp.lower_ap_dma(in_ap, for_custom_bir_dma=True)
    _idxs_ap = gp.lower_ap(idxs_ap)
    _out_ap = gp.lower_ap(out_ap)
    return gp.add_instruction(
        mybir.InstDMAGatherAnt(
            name=gp.bass.get_next_instruction_name(),
            ins=[*_in_ap, _idxs_ap, gp.lower_val_access(gp.to_reg(num_idxs))],
            outs=[_out_ap],
            transpose=False,
            num_idxs=num_idxs,
            elem_size=elem_size,
            stride_bytes_256=stride_bytes_256,
            gen_mode=0,
            single_packet=single_packet,
            queue_num=queue_num,
            sbuf_tokens_per_rank=0,
            sbuf_free_dim_per_rank=0,
            sbuf_free_dim_pad_per_rank=0,
            sbuf_byte_offset=0,
        )
    )


def _build(meta):
    import concourse.bacc as bacc
    import concourse.mybir as mybir
    import concourse.tile as tile

    g = meta["g"]
    nblk, npad, bpc = g["nblk"], g["npad"], g["bpc"]
    win, nwin, cb, nchunk = g["win"], g["nwin"], g["cb"], g["nchunk"]
    tbw, grp_off, ninst = meta["tbw"], meta["grp_off"], meta["ninst"]
    ktot, ttot = meta["ktot"], meta["ttot"]

    f32 = mybir.dt.float32
    bf16 = mybir.dt.bfloat16
    i16 = mybir.dt.int16

    nc = bacc.Bacc("TRN2", target_bir_lowering=False, debug=False,
                   num_swdge_queues=4)

    xb = nc.dram_tensor("xb", [npad, 2 * D], bf16, kind="ExternalInput")
    w_mat = nc.dram_tensor("w_mat", [D, D], f32, kind="ExternalInput")
    bias_t = nc.dram_tensor("bias_t", [BLK, D], f32, kind="ExternalInput")
    iota_d = nc.dram_tensor("iota", [BLK, PTB * BLK], bf16, kind="ExternalInput")
    idxs_d = nc.dram_tensor("idxs", [BLK, ktot // 16], i16, kind="ExternalInput")
    dstloc_d = nc.dram_tensor("dstloc", [BLK, ttot], bf16, kind="ExternalInput")
    dinv_own_d = nc.dram_tensor("dinv_own", [BLK, bpc], f32, kind="ExternalInput")
    out_d = nc.dram_tensor("out", [bpc * BLK, D], f32, kind="ExternalOutput")

    with tile.TileContext(nc) as tc:
        with (
            tc.tile_pool(name="const", bufs=1) as cpool,
            tc.tile_pool(name="mbuf", bufs=3) as mpool,
            tc.tile_pool(name="pt", bufs=8) as ptpool,
            tc.tile_pool(name="agg", bufs=4) as aggpool,
            tc.tile_pool(name="ob", bufs=4) as obpool,
            tc.tile_pool(name="ps1", bufs=2, space="PSUM") as ps1pool,
            tc.tile_pool(name="ps2", bufs=2, space="PSUM") as ps2pool,
        ):
            w_sb = cpool.tile([D, D], f32, tag="w")
            nc.sync.dma_start(out=w_sb[:], in_=w_mat[:])
            bias_sb = cpool.tile([BLK, D], f32, tag="bias")
            nc.sync.dma_start(out=bias_sb[:], in_=bias_t[:])
            iota_sb = cpool.tile([BLK, PTB * BLK], bf16, tag="iota")
            nc.sync.dma_start(out=iota_sb[:], in_=iota_d[:])
            idxs_sb = cpool.tile([BLK, ktot // 16], i16, tag="idxs")
            nc.sync.dma_start(out=idxs_sb[:], in_=idxs_d[:])
            dloc_sb = cpool.tile([BLK, ttot], bf16, tag="dloc")
            nc.sync.dma_start(out=dloc_sb[:], in_=dstloc_d[:])
            dinv_sb = cpool.tile([BLK, bpc], f32, tag="dinv")
            nc.sync.dma_start(out=dinv_sb[:], in_=dinv_own_d[:])

            # window row counts in the xb table
            wrows = [min(win, npad - w * win) for w in range(nwin)]

            inst_by_chunk = {}
            for (c, w, num_idxs, off) in ninst:
                inst_by_chunk.setdefault(c, []).append((w, num_idxs, off))

            # dma_gather descriptor budget: one desc per 16 idxs per engine;
            # cap each instruction well under the SWDGE ring capacity.
            GCAP = 3456
            qn = [0]

            def emit_gather(mt, w, off, num_idxs, es, row_lo):
                pos = 0
                while pos < num_idxs:
                    ni = min(GCAP, num_idxs - pos)
                    o = off + pos
                    _dma_gather_small(
                        nc.gpsimd,
                        mt[:, pos // BLK:(pos + ni) // BLK, :],
                        xb[w * win:w * win + wrows[w], row_lo:row_lo + es],
                        idxs_sb[:, o // 16:(o + ni) // 16],
                        ni,
                        es,
                        2 * D,
                        queue_num=qn[0] % 4,
                        single_packet=False,
                    )
                    qn[0] += 1
                    pos += ni

            for c in range(nchunk):
                mtiles = {}
                for (w, num_idxs, off) in inst_by_chunk[c]:
                    if num_idxs == 0:
                        continue
                    t_cw = num_idxs // BLK
                    if ELEM128:
                        mt = mpool.tile([BLK, t_cw, 2 * D], bf16, tag=f"m{w}")
                        emit_gather(mt, w, off, num_idxs, 2 * D, 0)
                    else:
                        mt = mpool.tile([BLK, t_cw, D], bf16, tag=f"m{w}")
                        emit_gather(mt, w, off, num_idxs, D, 0)
                    mtiles[w] = (mt, off)

                b_lo, b_hi = c * cb, min((c + 1) * cb, bpc)
                for lb in range(b_lo, b_hi):
                    tb = int(tbw[lb].sum())
                    ps = ps1pool.tile([D, BLK], f32, tag="agg")
                    k = 0
                    for w in range(nwin):
                        tB = int(tbw[lb, w])
                        col_b = grp_off[lb, w] // BLK
                        mt, moff = mtiles[w]
                        gt_b = (grp_off[lb, w] - moff) // BLK
                        for t0 in range(0, tB, PTB):
                            nb = min(PTB, tB - t0)
                            pt = ptpool.tile([BLK, PTB * BLK], bf16, tag="pt")
                            if nb == 1:
                                bc = (dloc_sb[:, col_b + t0:col_b + t0 + 1]
                                      .to_broadcast([BLK, BLK]))
                            else:
                                bc = (dloc_sb[:, col_b + t0:col_b + t0 + nb]
                                      .to_broadcast([BLK, nb, BLK]))
                            nc.vector.tensor_tensor(
                                out=pt[:, 0:nb * BLK],
                                in0=iota_sb[:, 0:nb * BLK],
                                in1=bc,
                                op=mybir.AluOpType.is_equal,
                            )
                            for j in range(nb):
                                nc.tensor.matmul(
                                    ps[:],
                                    mt[:, gt_b + t0 + j, 0:D],
                                    pt[:, j * BLK:(j + 1) * BLK],
                                    start=(k == 0),
                                    stop=(k == tb - 1),
                                )
                                k += 1
                    aggt = aggpool.tile([D, BLK], f32, tag="aggt")
                    nc.scalar.copy(out=aggt[:], in_=ps[:])
                    ps2 = ps2pool.tile([BLK, D], f32, tag="o2")
                    nc.tensor.matmul(ps2[:], aggt[:], w_sb[:], start=True, stop=True)
                    ob = obpool.tile([BLK, D], f32, tag="ob")
                    nc.vector.tensor_tensor(
                        out=ob[:], in0=ps2[:],
                        in1=dinv_sb[:, lb:lb + 1].to_broadcast([BLK, D]),
                        op=mybir.AluOpType.mult,
                    )
                    nc.vector.tensor_add(out=ob[:], in0=ob[:], in1=bias_sb[:])
                    nc.sync.dma_start(out=out_d[lb * BLK:(lb + 1) * BLK, :], in_=ob[:])
    return nc


# ---------------------------------------------------------------------------
# Entry points
# ---------------------------------------------------------------------------

def _install_ntff_hook_shim():
    """The agent image's antenv package lacks axon_hooks; provide it so
    run_bass_kernel_spmd(trace=True) can reach the NTFF profiler via the
    ctypes hook that trn_agent_boot carries."""
    import sys
    import types

    try:
        import antenv.axon_hooks  # noqa: F401
        return
    except ImportError:
        pass
    try:
        from trn_agent_boot.trn_boot import _ntff_profile_via_ctypes

        hook = _ntff_profile_via_ctypes("/opt/axon/libaxon_pjrt.so")
    except Exception:
        hook = None
    mod = types.ModuleType("antenv.axon_hooks")
    mod.get_axon_ntff_profile_hook = lambda: hook
    mod.set_axon_ntff_profile_hook = lambda h: None
    sys.modules["antenv.axon_hooks"] = mod


def _run(x, edge_index, W, b, trace=False):
    from concourse.bass_utils import run_bass_kernel_spmd

    if trace:
        _install_ntff_hook_shim()

    in_maps, meta = _prep(x, edge_index, W, b)
    nc = _build(meta)
    nc.compile()
    res = run_bass_kernel_spmd(
        nc, in_maps, list(range(meta["n_cores"])), trace=trace
    )
    n, n_cores = meta["n"], meta["n_cores"]
    g = meta["g"]
    perm = meta["perm"]
    out = np.empty((g["nblk"], BLK, D), dtype=np.float32)
    for m in range(n_cores):
        out[perm[m]] = res.results[m]["out"].reshape(g["bpc"], BLK, D)
    return out.reshape(-1, D)[:n].astype(np.float32), res


def kernel(x, edge_index, W, b):
    out, _ = _run(x, edge_index, W, b, trace=False)
    return out


def kernel_profiled(x, edge_index, W, b):
    out, res = _run(x, edge_index, W, b, trace=True)
    return out, res
